# revision 11
# baseline (speedup 1.0000x reference)
"""LocalAttention (B=1, S=4096, D=1024, H=16, hd=64, window=128) on 8 trn2 cores.

Sharding: sequence-parallel. Core c owns queries [512c, 512c+512) and receives
a key/value halo slice of 768 rows ([512c-128, 512c+640), zero-padded at the
global edges). All projection weights are replicated (bf16). Everything on
device runs in bf16 with fp32 PSUM accumulation.

Per-core dataflow:
  qT = (Wq^T x^T) in [e, s] layout   (lhsT=Wq slab, rhs=host-transposed Q)
  kT = same for K;  v = (Vin Wv) in natural [s, e] layout with an extra
      ones-column per head (for softmax denominators).
  scoresT[kt, qi] per (head, q-block of 128): 3 banded 128x128 matmuls into
      one PSUM tile [128, 384]; exp via ScalarE (scale=1/8, no max-subtract:
      scores are O(5)); triangle masks applied post-exp by DVE multiplies
      (only the two diagonal sub-tiles need masking; masks are per-core DATA
      so the SPMD program is identical on all cores).
  PV: out[sq, 65] = sum_r expPT_r^T @ [v_h | 1]; col 64 = softmax denom.
      Normalize with DVE reciprocal + tensor_scalar (per-partition).
  Transpose attn-out via TensorE (identity), then output projection
      o[s, :] = attnT^T @ Wo, fp32 out, DMA per 128-row block.
"""

import os

import numpy as np
import ml_dtypes

import concourse.bass as bass
import concourse.bacc as bacc
import concourse.mybir as mybir
import concourse.tile as tile
from concourse.bass_utils import run_bass_kernel_spmd

BF16 = mybir.dt.bfloat16
FP32 = mybir.dt.float32

NCORES = 8
S = 4096
D = 1024
H = 16
HD = 64
E = H * HD  # 1024
WIN = 128
SL = S // NCORES       # 512 queries per core
SK = SL + 2 * WIN      # 768 keys/values incl. halo
NQB = SL // 128        # 4 query blocks
NKB = SK // 128        # 6 key blocks
NDB = D // 128         # 8 contraction blocks
NEB = E // 128         # 8 embed blocks
VROW = HD + 1          # 65: v columns per head incl. ones column

_CACHE = {}
LAST_RESULT = None  # BassKernelResults of the most recent run (for test.py)


def _attend(nc, psum, ep, rp, qh, kh, v_sb, msk_sb, ao_sb, h, he, scale):
    """Banded local attention for one head: 4 q-blocks of 128 queries."""
    for qb in range(NQB):
        pscr = psum.tile([128, 384], FP32, tag="ps")
        for r in range(3):
            kb = qb + r
            nc.tensor.matmul(
                pscr[:, r * 128:(r + 1) * 128],
                lhsT=kh[:, he * SK + kb * 128: he * SK + (kb + 1) * 128],
                rhs=qh[:, he * SL + qb * 128: he * SL + (qb + 1) * 128],
                start=True,
                stop=True,
            )
        expp = ep.tile([128, 384], BF16, tag="expp")
        nc.scalar.activation(
            expp[:], pscr[:], mybir.ActivationFunctionType.Exp, scale=scale
        )
        m0 = msk_sb[:, (qb * 3 + 0) * 128:(qb * 3 + 1) * 128]
        m2 = msk_sb[:, (qb * 3 + 2) * 128:(qb * 3 + 3) * 128]
        nc.vector.tensor_mul(expp[:, 0:128], expp[:, 0:128], m0)
        nc.vector.tensor_mul(expp[:, 256:384], expp[:, 256:384], m2)

        ppv = psum.tile([128, VROW], FP32, tag="ps")
        for r in range(3):
            kb = qb + r
            nc.tensor.matmul(
                ppv[:],
                lhsT=expp[:, r * 128:(r + 1) * 128],
                rhs=v_sb[:, (kb * H + h) * VROW:(kb * H + h + 1) * VROW],
                start=(r == 0),
                stop=(r == 2),
            )
        rd = rp.tile([128, 1], FP32, tag="rd")
        nc.vector.reciprocal(rd[:], ppv[:, HD:VROW])
        nc.vector.tensor_scalar(
            ao_sb[:, qb * E + h * HD: qb * E + (h + 1) * HD],
            ppv[:, 0:HD],
            rd[:],
            None,
            op0=mybir.AluOpType.mult,
        )


def _build_nc():
    nc = bacc.Bacc("TRN2", target_bir_lowering=False, debug=False)

    qt_d = nc.dram_tensor("qt", [D, SL], BF16, kind="ExternalInput").ap()
    kt_d = nc.dram_tensor("kt", [D, SK], BF16, kind="ExternalInput").ap()
    vt_d = nc.dram_tensor("vt", [D, SK], BF16, kind="ExternalInput").ap()
    wq_d = nc.dram_tensor("wq", [D, E], BF16, kind="ExternalInput").ap()
    wk_d = nc.dram_tensor("wk", [D, E], BF16, kind="ExternalInput").ap()
    wv_d = nc.dram_tensor("wv", [D, E], BF16, kind="ExternalInput").ap()
    wo_d = nc.dram_tensor("wo", [E, D], BF16, kind="ExternalInput").ap()
    msk_d = nc.dram_tensor("msk", [NQB * 3, 128, 128], BF16, kind="ExternalInput").ap()
    idn_d = nc.dram_tensor("idn", [128, 128], BF16, kind="ExternalInput").ap()
    out_d = nc.dram_tensor("out", [SL, D], FP32, kind="ExternalOutput").ap()

    with tile.TileContext(nc) as tc:
        pools = []

        def pool(name, bufs, **kw):
            p = tc.tile_pool(name=name, bufs=bufs, **kw)
            pools.append(p)
            return p.__enter__()

        const = pool("const", 1)
        psum = pool("psum", 8, space="PSUM")
        ep = pool("expp", 6)
        rp = pool("recip", 4)

        # ---- persistent SBUF tensors ----
        wq_sb = const.tile([128, NDB * E], BF16, tag="wq")
        wk_sb = const.tile([128, NDB * E], BF16, tag="wk")
        wv_sb = const.tile([128, NDB * E], BF16, tag="wv")
        wo_sb = const.tile([128, NEB * D], BF16, tag="wo")
        qtin_sb = const.tile([128, NDB * SL], BF16, tag="qtin")
        ktin_sb = const.tile([128, NDB * SK], BF16, tag="ktin")
        vtin_sb = const.tile([128, NDB * SK], BF16, tag="vtin")
        qt_sb = const.tile([128, NEB * SL], BF16, tag="qt")    # [e,s] per e-blk
        kt_sb = const.tile([128, NEB * SK], BF16, tag="kt")
        v_sb = const.tile([128, NKB * H * VROW], BF16, tag="v")  # [s, h*65] per k-blk
        msk_sb = const.tile([128, NQB * 3 * 128], BF16, tag="msk")
        idn_sb = const.tile([128, 128], BF16, tag="idn")
        ao_sb = const.tile([128, NQB * E], BF16, tag="ao")     # attn out [sq, e]
        aot_sb = const.tile([128, NEB * SL], BF16, tag="aot")  # transposed [e, sq]
        o_sb = const.tile([128, NQB * D], FP32, tag="o")

        sync = nc.sync

        # ---- input DMAs: one batched transfer per tensor ----
        def load(sb, dr, ncols):
            sync.dma_start(
                sb[:].rearrange("p (b e) -> p b e", e=ncols),
                dr.rearrange("(b p) e -> p b e", p=128),
            )

        load(ktin_sb, kt_d, SK)
        load(wk_sb, wk_d, E)
        load(vtin_sb, vt_d, SK)
        load(wv_sb, wv_d, E)
        load(qtin_sb, qt_d, SL)
        load(wq_sb, wq_d, E)
        sync.dma_start(
            msk_sb[:].rearrange("p (m c) -> p m c", c=128),
            msk_d.rearrange("m p c -> p m c"),
        )
        sync.dma_start(idn_sb[:], idn_d[:])
        load(wo_sb, wo_d, D)

        # ones columns of v_sb (col hd=64 of each head group)
        v3 = v_sb[:].rearrange("p (k h c) -> p k h c", k=NKB, h=H)
        nc.gpsimd.memset(v3[:, :, :, HD:VROW], 1.0)

        # ---- k projection: [e, s] = Wk[d,e].T @ KT[d,s] ----
        for eb in range(NEB):
            for s0, s1 in ((0, 512), (512, SK)):
                ps = psum.tile([128, 512], FP32, tag="ps")
                for db in range(NDB):
                    nc.tensor.matmul(
                        ps[:, : s1 - s0],
                        lhsT=wk_sb[:, db * E + eb * 128: db * E + (eb + 1) * 128],
                        rhs=ktin_sb[:, db * SK + s0: db * SK + s1],
                        start=(db == 0),
                        stop=(db == NDB - 1),
                    )
                nc.vector.tensor_copy(
                    kt_sb[:, eb * SK + s0: eb * SK + s1], ps[:, : s1 - s0]
                )

        # ---- v projection (natural): [s, e] = VT[d,s].T @ Wv[d,e] ----
        for kb in range(NKB):
            for eh in range(2):
                ps = psum.tile([128, 512], FP32, tag="ps")
                for db in range(NDB):
                    nc.tensor.matmul(
                        ps[:],
                        lhsT=vtin_sb[:, db * SK + kb * 128: db * SK + (kb + 1) * 128],
                        rhs=wv_sb[:, db * E + eh * 512: db * E + (eh + 1) * 512],
                        start=(db == 0),
                        stop=(db == NDB - 1),
                    )
                dst = v3[:, kb, eh * 8:(eh + 1) * 8, 0:HD]
                src = ps[:].rearrange("p (h c) -> p h c", c=HD)
                nc.scalar.copy(dst, src)

        # ---- q projection interleaved with attention (keeps PE dense) ----
        scale = 1.0 / np.sqrt(HD)
        for eb in range(NEB):
            ps = psum.tile([128, 512], FP32, tag="ps")
            for db in range(NDB):
                nc.tensor.matmul(
                    ps[:],
                    lhsT=wq_sb[:, db * E + eb * 128: db * E + (eb + 1) * 128],
                    rhs=qtin_sb[:, db * SL: db * SL + SL],
                    start=(db == 0),
                    stop=(db == NDB - 1),
                )
            nc.vector.tensor_copy(qt_sb[:, eb * SL:(eb + 1) * SL], ps[:])

            for h in (2 * eb, 2 * eb + 1):
                hp = (h % 2) * HD          # partition offset inside e-block
                he = h // 2                # e-block of this head
                qh = qt_sb[hp:hp + HD]
                kh = kt_sb[hp:hp + HD]
                _attend(nc, psum, ep, rp, qh, kh, v_sb, msk_sb, ao_sb, h, he, scale)
        # ---- transpose attn-out: [sq, e] -> [e, sq] ----
        for qb in range(NQB):
            for eb in range(NEB):
                pt = psum.tile([128, 128], BF16, tag="ps")
                nc.tensor.transpose(
                    pt[:], ao_sb[:, qb * E + eb * 128: qb * E + (eb + 1) * 128], idn_sb[:]
                )
                nc.scalar.copy(
                    aot_sb[:, eb * SL + qb * 128: eb * SL + (qb + 1) * 128], pt[:]
                )

        # ---- output projection: o[s, :] = aoT[e, s].T @ Wo[e, :] ----
        for qb in range(NQB):
            for dh in range(2):
                ps = psum.tile([128, 512], FP32, tag="ps")
                for eb in range(NEB):
                    nc.tensor.matmul(
                        ps[:],
                        lhsT=aot_sb[:, eb * SL + qb * 128: eb * SL + (qb + 1) * 128],
                        rhs=wo_sb[:, eb * D + dh * 512: eb * D + (dh + 1) * 512],
                        start=(eb == 0),
                        stop=(eb == NEB - 1),
                    )
                nc.vector.tensor_copy(
                    o_sb[:, qb * D + dh * 512: qb * D + (dh + 1) * 512], ps[:]
                )
            sync.dma_start(
                out_d[qb * 128:(qb + 1) * 128, :], o_sb[:, qb * D:(qb + 1) * D]
            )

        for p in reversed(pools):
            p.__exit__(None, None, None)

    nc.compile()
    return nc


def _host_inputs(query, key, value, Wq, Wk, Wv, Wo):
    bf = ml_dtypes.bfloat16
    q2 = np.ascontiguousarray(query.reshape(S, D))
    k2 = np.asarray(key).reshape(S, D)
    v2 = np.asarray(value).reshape(S, D)
    kpad = np.zeros((S + 2 * WIN, D), np.float32)
    kpad[WIN:WIN + S] = k2
    vpad = np.zeros((S + 2 * WIN, D), np.float32)
    vpad[WIN:WIN + S] = v2

    wq = np.ascontiguousarray(Wq.astype(bf))
    wk = np.ascontiguousarray(Wk.astype(bf))
    wv = np.ascontiguousarray(Wv.astype(bf))
    wo = np.ascontiguousarray(Wo.astype(bf))
    idn = np.eye(128, dtype=bf)

    kt = np.arange(128)[:, None]
    qi = np.arange(128)[None, :]
    tri0 = (qi <= kt).astype(bf)
    tri2 = (kt <= qi).astype(bf)
    ones = np.ones((128, 128), bf)
    zeros = np.zeros((128, 128), bf)

    in_maps = []
    for c in range(NCORES):
        s0 = c * SL
        qt = np.ascontiguousarray(q2[s0:s0 + SL].T.astype(bf))
        ktc = np.ascontiguousarray(kpad[s0:s0 + SK].T.astype(bf))
        vtc = np.ascontiguousarray(vpad[s0:s0 + SK].T.astype(bf))
        msk = np.empty((NQB * 3, 128, 128), bf)
        for qb in range(NQB):
            m0 = zeros if (c == 0 and qb == 0) else tri0
            m2 = zeros if (c == NCORES - 1 and qb == NQB - 1) else tri2
            msk[qb * 3 + 0] = m0
            msk[qb * 3 + 1] = ones
            msk[qb * 3 + 2] = m2
        in_maps.append({
            "qt": qt, "kt": ktc, "vt": vtc,
            "wq": wq, "wk": wk, "wv": wv, "wo": wo,
            "msk": msk, "idn": idn,
        })
    return in_maps


def kernel(query, key, value, Wq, Wk, Wv, Wo):
    global LAST_RESULT
    if "nc" not in _CACHE:
        _CACHE["nc"] = _build_nc()
    nc = _CACHE["nc"]
    in_maps = _host_inputs(
        np.asarray(query), np.asarray(key), np.asarray(value),
        np.asarray(Wq), np.asarray(Wk), np.asarray(Wv), np.asarray(Wo),
    )
    trace = os.environ.get("KERNEL_TRACE", "0") == "1"
    try:
        res = run_bass_kernel_spmd(
            nc, in_maps, core_ids=list(range(NCORES)), trace=trace
        )
    except ModuleNotFoundError:
        res = run_bass_kernel_spmd(
            nc, in_maps, core_ids=list(range(NCORES)), trace=False
        )
    LAST_RESULT = res
    out = np.concatenate([res.results[c]["out"] for c in range(NCORES)], axis=0)
    return out.reshape(1, S, D).astype(np.float32)


# revision 12
# speedup vs baseline: 1.0753x; 1.0753x over previous
"""LocalAttention (B=1, S=4096, D=1024, H=16, hd=64, window=128) on 8 trn2 cores.

Sharding: sequence-parallel. Core c owns queries [512c, 512c+512) and receives
a key/value halo slice of 768 rows ([512c-128, 512c+640), zero-padded at the
global edges). All projection weights are replicated (bf16). Everything on
device runs in bf16 with fp32 PSUM accumulation.

Per-core dataflow:
  qT = (Wq^T x^T) in [e, s] layout   (lhsT=Wq slab, rhs=host-transposed Q)
  kT = same for K;  v = (Vin Wv) in natural [s, e] layout with an extra
      ones-column per head (for softmax denominators).
  scoresT[kt, qi] per (head, q-block of 128): 3 banded 128x128 matmuls into
      one PSUM tile [128, 384]; exp via ScalarE (scale=1/8, no max-subtract:
      scores are O(5)); triangle masks applied post-exp by DVE multiplies
      (only the two diagonal sub-tiles need masking; masks are per-core DATA
      so the SPMD program is identical on all cores).
  PV: out[sq, 65] = sum_r expPT_r^T @ [v_h | 1]; col 64 = softmax denom.
      Normalize with DVE reciprocal + tensor_scalar (per-partition).
  Transpose attn-out via TensorE (identity), then output projection
      o[s, :] = attnT^T @ Wo, fp32 out, DMA per 128-row block.
"""

import os

import numpy as np
import ml_dtypes

import concourse.bass as bass
import concourse.bacc as bacc
import concourse.mybir as mybir
import concourse.tile as tile
from concourse.bass_utils import run_bass_kernel_spmd

BF16 = mybir.dt.bfloat16
FP32 = mybir.dt.float32

NCORES = 8
S = 4096
D = 1024
H = 16
HD = 64
E = H * HD  # 1024
WIN = 128
SL = S // NCORES       # 512 queries per core
SK = SL + 2 * WIN      # 768 keys/values incl. halo
NQB = SL // 128        # 4 query blocks
NKB = SK // 128        # 6 key blocks
NDB = D // 128         # 8 contraction blocks
NEB = E // 128         # 8 embed blocks
VROW = HD + 1          # 65: v columns per head incl. ones column

_CACHE = {}
LAST_RESULT = None  # BassKernelResults of the most recent run (for test.py)


def _attend(nc, psum, ep, rp, qh, kh, v_sb, msk_sb, ao_sb, h, he, scale):
    """Banded local attention for one head: 4 q-blocks of 128 queries."""
    for qb in range(NQB):
        pscr = psum.tile([128, 384], FP32, tag="ps")
        for r in range(3):
            kb = qb + r
            nc.tensor.matmul(
                pscr[:, r * 128:(r + 1) * 128],
                lhsT=kh[:, he * SK + kb * 128: he * SK + (kb + 1) * 128],
                rhs=qh[:, he * SL + qb * 128: he * SL + (qb + 1) * 128],
                start=True,
                stop=True,
            )
        expp = ep.tile([128, 384], BF16, tag="expp")
        nc.scalar.activation(
            expp[:], pscr[:], mybir.ActivationFunctionType.Exp, scale=scale
        )
        m0 = msk_sb[:, (qb * 3 + 0) * 128:(qb * 3 + 1) * 128]
        m2 = msk_sb[:, (qb * 3 + 2) * 128:(qb * 3 + 3) * 128]
        nc.vector.tensor_mul(expp[:, 0:128], expp[:, 0:128], m0)
        nc.vector.tensor_mul(expp[:, 256:384], expp[:, 256:384], m2)

        ppv = psum.tile([128, VROW], FP32, tag="ps")
        for r in range(3):
            kb = qb + r
            nc.tensor.matmul(
                ppv[:],
                lhsT=expp[:, r * 128:(r + 1) * 128],
                rhs=v_sb[:, (kb * H + h) * VROW:(kb * H + h + 1) * VROW],
                start=(r == 0),
                stop=(r == 2),
            )
        rd = rp.tile([128, 1], FP32, tag="rd")
        nc.vector.reciprocal(rd[:], ppv[:, HD:VROW])
        nc.vector.tensor_scalar(
            ao_sb[:, qb * E + h * HD: qb * E + (h + 1) * HD],
            ppv[:, 0:HD],
            rd[:],
            None,
            op0=mybir.AluOpType.mult,
        )


def _build_nc():
    nc = bacc.Bacc("TRN2", target_bir_lowering=False, debug=False)

    qt_d = nc.dram_tensor("qt", [D, SL], BF16, kind="ExternalInput").ap()
    kt_d = nc.dram_tensor("kt", [D, SK], BF16, kind="ExternalInput").ap()
    vt_d = nc.dram_tensor("vt", [D, SK], BF16, kind="ExternalInput").ap()
    wq_d = nc.dram_tensor("wq", [D, E], BF16, kind="ExternalInput").ap()
    wk_d = nc.dram_tensor("wk", [D, E], BF16, kind="ExternalInput").ap()
    wv_d = nc.dram_tensor("wv", [D, E], BF16, kind="ExternalInput").ap()
    wo_d = nc.dram_tensor("wo", [E, D], BF16, kind="ExternalInput").ap()
    msk_d = nc.dram_tensor("msk", [NQB * 3, 128, 128], BF16, kind="ExternalInput").ap()
    idn_d = nc.dram_tensor("idn", [128, 128], BF16, kind="ExternalInput").ap()
    out_d = nc.dram_tensor("out", [SL, D], FP32, kind="ExternalOutput").ap()

    with tile.TileContext(nc) as tc:
        pools = []

        def pool(name, bufs, **kw):
            p = tc.tile_pool(name=name, bufs=bufs, **kw)
            pools.append(p)
            return p.__enter__()

        const = pool("const", 1)
        psum = pool("psum", 8, space="PSUM")
        ep = pool("expp", 4)
        rp = pool("recip", 4)

        # ---- persistent SBUF tensors ----
        wq_sb = const.tile([128, NDB * E], BF16, tag="wq")
        wk_sb = const.tile([128, NDB * E], BF16, tag="wk")
        wv_sb = const.tile([128, NDB * E], BF16, tag="wv")
        wo_sb = const.tile([128, NEB * D], BF16, tag="wo")
        qtin_sb = const.tile([128, NDB * SL], BF16, tag="qtin")
        ktin_sb = const.tile([128, NDB * SK], BF16, tag="ktin")
        vtin_sb = const.tile([128, NDB * SK], BF16, tag="vtin")
        qt_sb = const.tile([128, NEB * SL], BF16, tag="qt")    # [e,s] per e-blk
        kt_sb = const.tile([128, NEB * SK], BF16, tag="kt")
        v_sb = const.tile([128, NKB * H * VROW], BF16, tag="v")  # [s, h*65] per k-blk
        msk_sb = const.tile([128, NQB * 3 * 128], BF16, tag="msk")
        idn_sb = const.tile([128, 128], BF16, tag="idn")
        ao_sb = const.tile([128, NQB * E], BF16, tag="ao")     # attn out [sq, e]
        aot_sb = const.tile([128, NEB * SL], BF16, tag="aot")  # transposed [e, sq]
        o_sb = const.tile([128, NQB * D], FP32, tag="o")

        sync = nc.sync

        # ---- input DMAs: one batched transfer per tensor ----
        def load(sb, dr, ncols):
            sync.dma_start(
                sb[:].rearrange("p (b e) -> p b e", e=ncols),
                dr.rearrange("(b p) e -> p b e", p=128),
            )

        load(qtin_sb, qt_d, SL)
        load(wq_sb, wq_d, E)
        load(ktin_sb, kt_d, SK)
        load(wk_sb, wk_d, E)
        load(vtin_sb, vt_d, SK)
        load(wv_sb, wv_d, E)
        sync.dma_start(
            msk_sb[:].rearrange("p (m c) -> p m c", c=128),
            msk_d.rearrange("m p c -> p m c"),
        )
        sync.dma_start(idn_sb[:], idn_d[:])
        load(wo_sb, wo_d, D)

        # ones columns of v_sb (col hd=64 of each head group)
        v3 = v_sb[:].rearrange("p (k h c) -> p k h c", k=NKB, h=H)
        nc.gpsimd.memset(v3[:, :, :, HD:VROW], 1.0)

        # ---- q projection ----
        for eb in range(NEB):
            ps = psum.tile([128, 512], FP32, tag="ps")
            for db in range(NDB):
                nc.tensor.matmul(
                    ps[:],
                    lhsT=wq_sb[:, db * E + eb * 128: db * E + (eb + 1) * 128],
                    rhs=qtin_sb[:, db * SL: db * SL + SL],
                    start=(db == 0),
                    stop=(db == NDB - 1),
                )
            nc.vector.tensor_copy(qt_sb[:, eb * SL:(eb + 1) * SL], ps[:])

        # ---- k projection: [e, s] = Wk[d,e].T @ KT[d,s] ----
        for eb in range(NEB):
            for s0, s1 in ((0, 512), (512, SK)):
                ps = psum.tile([128, 512], FP32, tag="ps")
                for db in range(NDB):
                    nc.tensor.matmul(
                        ps[:, : s1 - s0],
                        lhsT=wk_sb[:, db * E + eb * 128: db * E + (eb + 1) * 128],
                        rhs=ktin_sb[:, db * SK + s0: db * SK + s1],
                        start=(db == 0),
                        stop=(db == NDB - 1),
                    )
                nc.vector.tensor_copy(
                    kt_sb[:, eb * SK + s0: eb * SK + s1], ps[:, : s1 - s0]
                )

        # ---- v projection (natural): [s, e] = VT[d,s].T @ Wv[d,e] ----
        for kb in range(NKB):
            for eh in range(2):
                ps = psum.tile([128, 512], FP32, tag="ps")
                for db in range(NDB):
                    nc.tensor.matmul(
                        ps[:],
                        lhsT=vtin_sb[:, db * SK + kb * 128: db * SK + (kb + 1) * 128],
                        rhs=wv_sb[:, db * E + eh * 512: db * E + (eh + 1) * 512],
                        start=(db == 0),
                        stop=(db == NDB - 1),
                    )
                dst = v3[:, kb, eh * 8:(eh + 1) * 8, 0:HD]
                src = ps[:].rearrange("p (h c) -> p h c", c=HD)
                nc.scalar.copy(dst, src)

        # ---- attention ----
        scale = 1.0 / np.sqrt(HD)
        for h in range(H):
            hp = (h % 2) * HD          # partition offset inside e-block
            he = h // 2                # e-block of this head
            qh = qt_sb[hp:hp + HD]
            kh = kt_sb[hp:hp + HD]
            _attend(nc, psum, ep, rp, qh, kh, v_sb, msk_sb, ao_sb, h, he, scale)
        # ---- transpose attn-out: [sq, e] -> [e, sq] ----
        for qb in range(NQB):
            for eb in range(NEB):
                pt = psum.tile([128, 128], BF16, tag="ps")
                nc.tensor.transpose(
                    pt[:], ao_sb[:, qb * E + eb * 128: qb * E + (eb + 1) * 128], idn_sb[:]
                )
                nc.scalar.copy(
                    aot_sb[:, eb * SL + qb * 128: eb * SL + (qb + 1) * 128], pt[:]
                )

        # ---- output projection: o[s, :] = aoT[e, s].T @ Wo[e, :] ----
        for qb in range(NQB):
            for dh in range(2):
                ps = psum.tile([128, 512], FP32, tag="ps")
                for eb in range(NEB):
                    nc.tensor.matmul(
                        ps[:],
                        lhsT=aot_sb[:, eb * SL + qb * 128: eb * SL + (qb + 1) * 128],
                        rhs=wo_sb[:, eb * D + dh * 512: eb * D + (dh + 1) * 512],
                        start=(eb == 0),
                        stop=(eb == NEB - 1),
                    )
                nc.vector.tensor_copy(
                    o_sb[:, qb * D + dh * 512: qb * D + (dh + 1) * 512], ps[:]
                )
            sync.dma_start(
                out_d[qb * 128:(qb + 1) * 128, :], o_sb[:, qb * D:(qb + 1) * D]
            )

        for p in reversed(pools):
            p.__exit__(None, None, None)

    nc.compile()
    return nc


def _host_inputs(query, key, value, Wq, Wk, Wv, Wo):
    bf = ml_dtypes.bfloat16
    q2 = np.ascontiguousarray(query.reshape(S, D))
    k2 = np.asarray(key).reshape(S, D)
    v2 = np.asarray(value).reshape(S, D)
    kpad = np.zeros((S + 2 * WIN, D), np.float32)
    kpad[WIN:WIN + S] = k2
    vpad = np.zeros((S + 2 * WIN, D), np.float32)
    vpad[WIN:WIN + S] = v2

    wq = np.ascontiguousarray(Wq.astype(bf))
    wk = np.ascontiguousarray(Wk.astype(bf))
    wv = np.ascontiguousarray(Wv.astype(bf))
    wo = np.ascontiguousarray(Wo.astype(bf))
    idn = np.eye(128, dtype=bf)

    kt = np.arange(128)[:, None]
    qi = np.arange(128)[None, :]
    tri0 = (qi <= kt).astype(bf)
    tri2 = (kt <= qi).astype(bf)
    ones = np.ones((128, 128), bf)
    zeros = np.zeros((128, 128), bf)

    in_maps = []
    for c in range(NCORES):
        s0 = c * SL
        qt = np.ascontiguousarray(q2[s0:s0 + SL].T.astype(bf))
        ktc = np.ascontiguousarray(kpad[s0:s0 + SK].T.astype(bf))
        vtc = np.ascontiguousarray(vpad[s0:s0 + SK].T.astype(bf))
        msk = np.empty((NQB * 3, 128, 128), bf)
        for qb in range(NQB):
            m0 = zeros if (c == 0 and qb == 0) else tri0
            m2 = zeros if (c == NCORES - 1 and qb == NQB - 1) else tri2
            msk[qb * 3 + 0] = m0
            msk[qb * 3 + 1] = ones
            msk[qb * 3 + 2] = m2
        in_maps.append({
            "qt": qt, "kt": ktc, "vt": vtc,
            "wq": wq, "wk": wk, "wv": wv, "wo": wo,
            "msk": msk, "idn": idn,
        })
    return in_maps


def kernel(query, key, value, Wq, Wk, Wv, Wo):
    global LAST_RESULT
    if "nc" not in _CACHE:
        _CACHE["nc"] = _build_nc()
    nc = _CACHE["nc"]
    in_maps = _host_inputs(
        np.asarray(query), np.asarray(key), np.asarray(value),
        np.asarray(Wq), np.asarray(Wk), np.asarray(Wv), np.asarray(Wo),
    )
    trace = os.environ.get("KERNEL_TRACE", "0") == "1"
    try:
        res = run_bass_kernel_spmd(
            nc, in_maps, core_ids=list(range(NCORES)), trace=trace
        )
    except ModuleNotFoundError:
        res = run_bass_kernel_spmd(
            nc, in_maps, core_ids=list(range(NCORES)), trace=False
        )
    LAST_RESULT = res
    out = np.concatenate([res.results[c]["out"] for c in range(NCORES)], axis=0)
    return out.reshape(1, S, D).astype(np.float32)


# revision 13
# speedup vs baseline: 1.0974x; 1.0206x over previous
"""LocalAttention (B=1, S=4096, D=1024, H=16, hd=64, window=128) on 8 trn2 cores.

Sharding: sequence-parallel. Core c owns queries [512c, 512c+512) and receives
a key/value halo slice of 768 rows ([512c-128, 512c+640), zero-padded at the
global edges). All projection weights are replicated (bf16). Everything on
device runs in bf16 with fp32 PSUM accumulation.

Per-core dataflow:
  qT = (Wq^T x^T) in [e, s] layout   (lhsT=Wq slab, rhs=host-transposed Q)
  kT = same for K;  v = (Vin Wv) in natural [s, e] layout with an extra
      ones-column per head (for softmax denominators).
  scoresT[kt, qi] per (head, q-block of 128): 3 banded 128x128 matmuls into
      one PSUM tile [128, 384]; exp via ScalarE (scale=1/8, no max-subtract:
      scores are O(5)); triangle masks applied post-exp by DVE multiplies
      (only the two diagonal sub-tiles need masking; masks are per-core DATA
      so the SPMD program is identical on all cores).
  PV: out[sq, 65] = sum_r expPT_r^T @ [v_h | 1]; col 64 = softmax denom.
      Normalize with DVE reciprocal + tensor_scalar (per-partition).
  Transpose attn-out via TensorE (identity), then output projection
      o[s, :] = attnT^T @ Wo, fp32 out, DMA per 128-row block.
"""

import os

import numpy as np
import ml_dtypes

import concourse.bass as bass
import concourse.bacc as bacc
import concourse.mybir as mybir
import concourse.tile as tile
from concourse.bass_utils import run_bass_kernel_spmd

BF16 = mybir.dt.bfloat16
FP32 = mybir.dt.float32

NCORES = 8
S = 4096
D = 1024
H = 16
HD = 64
E = H * HD  # 1024
WIN = 128
SL = S // NCORES       # 512 queries per core
SK = SL + 2 * WIN      # 768 keys/values incl. halo
NQB = SL // 128        # 4 query blocks
NKB = SK // 128        # 6 key blocks
NDB = D // 128         # 8 contraction blocks
NEB = E // 128         # 8 embed blocks
VROW = HD + 1          # 65: v columns per head incl. ones column

_CACHE = {}
LAST_RESULT = None  # BassKernelResults of the most recent run (for test.py)


def _attend(nc, psum, ep, rp, qh, kh, v_sb, msk_sb, ao_sb, h, he, scale):
    """Banded local attention for one head: 4 q-blocks of 128 queries."""
    for qb in range(NQB):
        pscr = psum.tile([128, 384], FP32, tag="ps")
        for r in range(3):
            kb = qb + r
            nc.tensor.matmul(
                pscr[:, r * 128:(r + 1) * 128],
                lhsT=kh[:, he * SK + kb * 128: he * SK + (kb + 1) * 128],
                rhs=qh[:, he * SL + qb * 128: he * SL + (qb + 1) * 128],
                start=True,
                stop=True,
            )
        expp = ep.tile([128, 384], BF16, tag="expp")
        nc.scalar.activation(
            expp[:], pscr[:], mybir.ActivationFunctionType.Exp, scale=scale
        )
        m0 = msk_sb[:, (qb * 3 + 0) * 128:(qb * 3 + 1) * 128]
        m2 = msk_sb[:, (qb * 3 + 2) * 128:(qb * 3 + 3) * 128]
        nc.vector.tensor_mul(expp[:, 0:128], expp[:, 0:128], m0)
        nc.vector.tensor_mul(expp[:, 256:384], expp[:, 256:384], m2)

        ppv = psum.tile([128, VROW], FP32, tag="ps")
        for r in range(3):
            kb = qb + r
            nc.tensor.matmul(
                ppv[:],
                lhsT=expp[:, r * 128:(r + 1) * 128],
                rhs=v_sb[:, (kb * H + h) * VROW:(kb * H + h + 1) * VROW],
                start=(r == 0),
                stop=(r == 2),
            )
        rd = rp.tile([128, 1], FP32, tag="rd")
        nc.vector.reciprocal(rd[:], ppv[:, HD:VROW])
        nc.vector.tensor_scalar(
            ao_sb[:, qb * E + h * HD: qb * E + (h + 1) * HD],
            ppv[:, 0:HD],
            rd[:],
            None,
            op0=mybir.AluOpType.mult,
        )


def _build_nc():
    nc = bacc.Bacc("TRN2", target_bir_lowering=False, debug=False)

    qt_d = nc.dram_tensor("qt", [D, SL], BF16, kind="ExternalInput").ap()
    kt_d = nc.dram_tensor("kt", [D, SK], BF16, kind="ExternalInput").ap()
    vt_d = nc.dram_tensor("vt", [D, SK], BF16, kind="ExternalInput").ap()
    wq_d = nc.dram_tensor("wq", [D, E], BF16, kind="ExternalInput").ap()
    wk_d = nc.dram_tensor("wk", [D, E], BF16, kind="ExternalInput").ap()
    wv_d = nc.dram_tensor("wv", [D, E], BF16, kind="ExternalInput").ap()
    wo_d = nc.dram_tensor("wo", [E, D], BF16, kind="ExternalInput").ap()
    msk_d = nc.dram_tensor("msk", [NQB * 3, 128, 128], BF16, kind="ExternalInput").ap()
    idn_d = nc.dram_tensor("idn", [128, 128], BF16, kind="ExternalInput").ap()
    out_d = nc.dram_tensor("out", [SL, D], FP32, kind="ExternalOutput").ap()

    with tile.TileContext(nc) as tc:
        pools = []

        def pool(name, bufs, **kw):
            p = tc.tile_pool(name=name, bufs=bufs, **kw)
            pools.append(p)
            return p.__enter__()

        const = pool("const", 1)
        psum = pool("psum", 8, space="PSUM")
        ep = pool("expp", 6)
        rp = pool("recip", 8)

        # ---- persistent SBUF tensors ----
        wq_sb = const.tile([128, NDB * E], BF16, tag="wq")
        wk_sb = const.tile([128, NDB * E], BF16, tag="wk")
        wv_sb = const.tile([128, NDB * E], BF16, tag="wv")
        wo_sb = const.tile([128, NEB * D], BF16, tag="wo")
        qtin_sb = const.tile([128, NDB * SL], BF16, tag="qtin")
        ktin_sb = const.tile([128, NDB * SK], BF16, tag="ktin")
        vtin_sb = const.tile([128, NDB * SK], BF16, tag="vtin")
        qt_sb = const.tile([128, NEB * SL], BF16, tag="qt")    # [e,s] per e-blk
        kt_sb = const.tile([128, NEB * SK], BF16, tag="kt")
        v_sb = const.tile([128, NKB * H * VROW], BF16, tag="v")  # [s, h*65] per k-blk
        msk_sb = const.tile([128, NQB * 3 * 128], BF16, tag="msk")
        idn_sb = const.tile([128, 128], BF16, tag="idn")
        ao_sb = const.tile([128, NQB * E], BF16, tag="ao")     # attn out [sq, e]
        aot_sb = const.tile([128, NEB * SL], BF16, tag="aot")  # transposed [e, sq]
        o_sb = const.tile([128, NQB * D], FP32, tag="o")

        sync = nc.sync

        # ---- input DMAs: one batched transfer per tensor ----
        def load(sb, dr, ncols):
            sync.dma_start(
                sb[:].rearrange("p (b e) -> p b e", e=ncols),
                dr.rearrange("(b p) e -> p b e", p=128),
            )

        def load_half(sb, dr, ncols, half, w):
            sync.dma_start(
                sb[:, half * 4 * ncols:(half * 4 + 4) * ncols].rearrange(
                    "p (b e) -> p b e", e=ncols
                ),
                dr[half * 4 * 128:(half * 4 + 4) * 128].rearrange(
                    "(b p) e -> p b e", p=128
                ),
            )

        load_half(qtin_sb, qt_d, SL, 0, None)
        load_half(wq_sb, wq_d, E, 0, None)
        load_half(qtin_sb, qt_d, SL, 1, None)
        load_half(wq_sb, wq_d, E, 1, None)
        load(ktin_sb, kt_d, SK)
        load(wk_sb, wk_d, E)
        load(vtin_sb, vt_d, SK)
        load(wv_sb, wv_d, E)
        sync.dma_start(
            msk_sb[:].rearrange("p (m c) -> p m c", c=128),
            msk_d.rearrange("m p c -> p m c"),
        )
        sync.dma_start(idn_sb[:], idn_d[:])
        load(wo_sb, wo_d, D)

        # ones columns of v_sb (col hd=64 of each head group)
        v3 = v_sb[:].rearrange("p (k h c) -> p k h c", k=NKB, h=H)
        nc.gpsimd.memset(v3[:, :, :, HD:VROW], 1.0)

        # ---- q projection ----
        for eb in range(NEB):
            ps = psum.tile([128, 512], FP32, tag="ps")
            for db in range(NDB):
                nc.tensor.matmul(
                    ps[:],
                    lhsT=wq_sb[:, db * E + eb * 128: db * E + (eb + 1) * 128],
                    rhs=qtin_sb[:, db * SL: db * SL + SL],
                    start=(db == 0),
                    stop=(db == NDB - 1),
                )
            nc.vector.tensor_copy(qt_sb[:, eb * SL:(eb + 1) * SL], ps[:])

        # ---- k projection: [e, s] = Wk[d,e].T @ KT[d,s] ----
        for eb in range(NEB):
            for s0, s1 in ((0, 512), (512, SK)):
                ps = psum.tile([128, 512], FP32, tag="ps")
                for db in range(NDB):
                    nc.tensor.matmul(
                        ps[:, : s1 - s0],
                        lhsT=wk_sb[:, db * E + eb * 128: db * E + (eb + 1) * 128],
                        rhs=ktin_sb[:, db * SK + s0: db * SK + s1],
                        start=(db == 0),
                        stop=(db == NDB - 1),
                    )
                nc.vector.tensor_copy(
                    kt_sb[:, eb * SK + s0: eb * SK + s1], ps[:, : s1 - s0]
                )

        # ---- v projection (natural): [s, e] = VT[d,s].T @ Wv[d,e] ----
        for kb in range(NKB):
            for eh in range(2):
                ps = psum.tile([128, 512], FP32, tag="ps")
                for db in range(NDB):
                    nc.tensor.matmul(
                        ps[:],
                        lhsT=vtin_sb[:, db * SK + kb * 128: db * SK + (kb + 1) * 128],
                        rhs=wv_sb[:, db * E + eh * 512: db * E + (eh + 1) * 512],
                        start=(db == 0),
                        stop=(db == NDB - 1),
                    )
                dst = v3[:, kb, eh * 8:(eh + 1) * 8, 0:HD]
                src = ps[:].rearrange("p (h c) -> p h c", c=HD)
                nc.scalar.copy(dst, src)

        # ---- attention ----
        scale = 1.0 / np.sqrt(HD)
        for h in range(H):
            hp = (h % 2) * HD          # partition offset inside e-block
            he = h // 2                # e-block of this head
            qh = qt_sb[hp:hp + HD]
            kh = kt_sb[hp:hp + HD]
            _attend(nc, psum, ep, rp, qh, kh, v_sb, msk_sb, ao_sb, h, he, scale)
        # ---- transpose attn-out: [sq, e] -> [e, sq] ----
        for qb in range(NQB):
            for eb in range(NEB):
                pt = psum.tile([128, 128], BF16, tag="ps")
                nc.tensor.transpose(
                    pt[:], ao_sb[:, qb * E + eb * 128: qb * E + (eb + 1) * 128], idn_sb[:]
                )
                nc.scalar.copy(
                    aot_sb[:, eb * SL + qb * 128: eb * SL + (qb + 1) * 128], pt[:]
                )

        # ---- output projection: o[s, :] = aoT[e, s].T @ Wo[e, :] ----
        for qb in range(NQB):
            for dh in range(2):
                ps = psum.tile([128, 512], FP32, tag="ps")
                for eb in range(NEB):
                    nc.tensor.matmul(
                        ps[:],
                        lhsT=aot_sb[:, eb * SL + qb * 128: eb * SL + (qb + 1) * 128],
                        rhs=wo_sb[:, eb * D + dh * 512: eb * D + (dh + 1) * 512],
                        start=(eb == 0),
                        stop=(eb == NEB - 1),
                    )
                nc.vector.tensor_copy(
                    o_sb[:, qb * D + dh * 512: qb * D + (dh + 1) * 512], ps[:]
                )
            sync.dma_start(
                out_d[qb * 128:(qb + 1) * 128, :], o_sb[:, qb * D:(qb + 1) * D]
            )

        for p in reversed(pools):
            p.__exit__(None, None, None)

    nc.compile()
    return nc


def _host_inputs(query, key, value, Wq, Wk, Wv, Wo):
    bf = ml_dtypes.bfloat16
    q2 = np.ascontiguousarray(query.reshape(S, D))
    k2 = np.asarray(key).reshape(S, D)
    v2 = np.asarray(value).reshape(S, D)
    kpad = np.zeros((S + 2 * WIN, D), np.float32)
    kpad[WIN:WIN + S] = k2
    vpad = np.zeros((S + 2 * WIN, D), np.float32)
    vpad[WIN:WIN + S] = v2

    wq = np.ascontiguousarray(Wq.astype(bf))
    wk = np.ascontiguousarray(Wk.astype(bf))
    wv = np.ascontiguousarray(Wv.astype(bf))
    wo = np.ascontiguousarray(Wo.astype(bf))
    idn = np.eye(128, dtype=bf)

    kt = np.arange(128)[:, None]
    qi = np.arange(128)[None, :]
    tri0 = (qi <= kt).astype(bf)
    tri2 = (kt <= qi).astype(bf)
    ones = np.ones((128, 128), bf)
    zeros = np.zeros((128, 128), bf)

    in_maps = []
    for c in range(NCORES):
        s0 = c * SL
        qt = np.ascontiguousarray(q2[s0:s0 + SL].T.astype(bf))
        ktc = np.ascontiguousarray(kpad[s0:s0 + SK].T.astype(bf))
        vtc = np.ascontiguousarray(vpad[s0:s0 + SK].T.astype(bf))
        msk = np.empty((NQB * 3, 128, 128), bf)
        for qb in range(NQB):
            m0 = zeros if (c == 0 and qb == 0) else tri0
            m2 = zeros if (c == NCORES - 1 and qb == NQB - 1) else tri2
            msk[qb * 3 + 0] = m0
            msk[qb * 3 + 1] = ones
            msk[qb * 3 + 2] = m2
        in_maps.append({
            "qt": qt, "kt": ktc, "vt": vtc,
            "wq": wq, "wk": wk, "wv": wv, "wo": wo,
            "msk": msk, "idn": idn,
        })
    return in_maps


def kernel(query, key, value, Wq, Wk, Wv, Wo):
    global LAST_RESULT
    if "nc" not in _CACHE:
        _CACHE["nc"] = _build_nc()
    nc = _CACHE["nc"]
    in_maps = _host_inputs(
        np.asarray(query), np.asarray(key), np.asarray(value),
        np.asarray(Wq), np.asarray(Wk), np.asarray(Wv), np.asarray(Wo),
    )
    trace = os.environ.get("KERNEL_TRACE", "0") == "1"
    try:
        res = run_bass_kernel_spmd(
            nc, in_maps, core_ids=list(range(NCORES)), trace=trace
        )
    except ModuleNotFoundError:
        res = run_bass_kernel_spmd(
            nc, in_maps, core_ids=list(range(NCORES)), trace=False
        )
    LAST_RESULT = res
    out = np.concatenate([res.results[c]["out"] for c in range(NCORES)], axis=0)
    return out.reshape(1, S, D).astype(np.float32)
